# revision 1
# baseline (speedup 1.0000x reference)
"""Bass/Trainium2 kernel for nn_BitwiseBasicBlock.

Computes (reference semantics, NCHW):
    out1 = BN(conv3x3(sign(x), sign(w1)*alpha1), g1, b1)     # training-mode BN
    out2 = BN(conv3x3(sign(out1), sign(w2)*alpha2), g2, b2)
    out  = out2 + x

Strategy:
  - Data-parallel over batch: 32 images -> 8 cores x 4 images. Weights replicated.
  - Sync-BN: per-core per-channel (sum, sumsq) of the integer conv outputs are
    AllReduce'd (2KB payload) so BN stats match the full-batch reference.
  - The conv operands are all +-1, so the 3x3 conv is 9 accumulated matmuls over
    a zero-padded 58-wide activation layout, run in fp8 (exact for +-1) with
    DoubleRow (contracts both 128-channel halves per instruction), accumulating
    exact integers in fp32 PSUM.
  - alpha (per-out-channel |w| mean) and the BN affine fold into a single
    per-channel scale+bias; for layer 1 it is fused with the sign binarization
    feeding layer 2 (one Scalar-engine activation op).
"""

import os
import sys

import numpy as np

for _p in ("/opt/trn_rl_repo",):
    if _p not in sys.path and os.path.isdir(_p):
        sys.path.insert(0, _p)

import ml_dtypes
from contextlib import ExitStack

import concourse.bass as bass
import concourse.tile as tile
from concourse import bacc
from concourse import mybir
from concourse.bass_utils import run_bass_kernel_spmd

F32 = mybir.dt.float32
F16 = mybir.dt.float16
BF16 = mybir.dt.bfloat16
F8 = mybir.dt.float8e4
F8NP = ml_dtypes.float8_e4m3
BF16NP = ml_dtypes.bfloat16

EPS = 1e-5
H = W = 56
PW = H + 2            # padded width
RPT = 8               # output rows per psum tile
NYC = H // RPT        # 7 row-chunks
NT = RPT * PW         # 464 <= 512 (one PSUM bank)
CH = 128              # channel chunk (partition dim)
PLANE_F = 3488        # per-chunk padded plane size; mult of 16, >= 58*58+2
OFFS = [(dy, dx) for dy in range(3) for dx in range(3)]

USE_FP8 = True        # fp8+DoubleRow (9 MM/tile) vs bf16 (18 MM/tile)
USE_COLLECTIVE = True # AllReduce for BN stats (False: local copy, 1-core only)

N_CORES = 8
N_IMG = 4             # images per core on HW


def build_nc(n_img, n_cores, stage=6):
    nc = bacc.Bacc("TRN2", target_bir_lowering=False)
    x_in = nc.dram_tensor("x", [n_img, 2 * CH, H, W], F32, kind="ExternalInput")
    wdt = F8 if USE_FP8 else BF16
    w1p = nc.dram_tensor("w1p", [CH, 9, 2, 2, CH], wdt, kind="ExternalInput")
    w2p = nc.dram_tensor("w2p", [CH, 9, 2, 2, CH], wdt, kind="ExternalInput")
    # aux cols: conv*6 + param*2 + oc, params (alpha, gamma, beta)
    aux = nc.dram_tensor("aux", [CH, 12], F32, kind="ExternalInput")
    out_t = nc.dram_tensor("out", [n_img, 2 * CH, H, W], F32, kind="ExternalOutput")

    count_g = float(n_img * n_cores * H * W)  # global per-channel element count
    pdt = F8 if USE_FP8 else BF16

    with ExitStack() as ctx:
        tc = ctx.enter_context(tile.TileContext(nc))
        singles = ctx.enter_context(tc.tile_pool(name="singles", bufs=1))
        xpool = ctx.enter_context(tc.tile_pool(name="xpool", bufs=6))
        planep = ctx.enter_context(tc.tile_pool(name="planep", bufs=3))
        spool = ctx.enter_context(tc.tile_pool(name="spool", bufs=2 * n_img + 2))
        statsp = ctx.enter_context(tc.tile_pool(name="statsp", bufs=1))
        coefp = ctx.enter_context(tc.tile_pool(name="coefp", bufs=1))
        psum = ctx.enter_context(tc.tile_pool(name="psum", bufs=8, space="PSUM"))
        fpool = ctx.enter_context(tc.tile_pool(name="fpool", bufs=6))
        xidp = ctx.enter_context(tc.tile_pool(name="xidp", bufs=8))
        dramp = ctx.enter_context(tc.tile_pool(name="dramp", bufs=2, space="DRAM"))
        scrp = ctx.enter_context(tc.tile_pool(name="scrp", bufs=3))

        w1t = singles.tile([CH, 9, 2, 2, CH], wdt)
        nc.sync.dma_start(out=w1t[:], in_=w1p[:])
        w2t = singles.tile([CH, 9, 2, 2, CH], wdt)
        nc.sync.dma_start(out=w2t[:], in_=w2p[:])
        auxt = singles.tile([CH, 12], F32)
        nc.sync.dma_start(out=auxt[:], in_=aux[:])
        epst = singles.tile([CH, 1], F32)
        nc.vector.memset(epst[:], EPS)

        def conv_tile(wt, plane, oc, yc, ps):
            """9 (or 18) accumulated matmuls into psum tile ps."""
            if USE_FP8:
                for k in range(9):
                    dy, dx = OFFS[k]
                    off = yc * RPT * PW + dy * PW + dx
                    nc.tensor.matmul(
                        out=ps[:],
                        lhsT=wt[:, k, oc],
                        rhs=plane[:, :, off : off + NT],
                        start=(k == 0),
                        stop=(k == 8),
                        perf_mode=mybir.MatmulPerfMode.DoubleRow,
                    )
            else:
                nmm = 18
                i = 0
                for k in range(9):
                    dy, dx = OFFS[k]
                    off = yc * RPT * PW + dy * PW + dx
                    for j in range(2):
                        nc.tensor.matmul(
                            out=ps[:],
                            lhsT=wt[:, k, oc, j],
                            rhs=plane[:, j, off : off + NT],
                            start=(i == 0),
                            stop=(i == nmm - 1),
                        )
                        i += 1

        def make_coefs(ccg, conv):
            """Global (sum, sumsq) of integer conv outputs -> per-channel A, B:
            BN(alpha*S)*g + b == S*A + B."""
            A, B = [], []
            for oc in range(2):
                al = auxt[:, (conv * 6 + 0 * 2 + oc) : (conv * 6 + 0 * 2 + oc) + 1]
                ga = auxt[:, (conv * 6 + 1 * 2 + oc) : (conv * 6 + 1 * 2 + oc) + 1]
                be = auxt[:, (conv * 6 + 2 * 2 + oc) : (conv * 6 + 2 * 2 + oc) + 1]
                m = coefp.tile([CH, 1], F32, tag=f"m{conv}{oc}", name=f"m{conv}{oc}")
                nc.vector.tensor_scalar_mul(m[:], ccg[:, 2 * oc : 2 * oc + 1], 1.0 / count_g)
                e2 = coefp.tile([CH, 1], F32, tag=f"e2{conv}{oc}", name=f"e2{conv}{oc}")
                nc.vector.tensor_scalar_mul(e2[:], ccg[:, 2 * oc + 1 : 2 * oc + 2], 1.0 / count_g)
                var = coefp.tile([CH, 1], F32, tag=f"var{conv}{oc}", name=f"var{conv}{oc}")
                nc.vector.tensor_mul(var[:], m[:], m[:])          # mean^2
                nc.vector.tensor_sub(var[:], e2[:], var[:])       # E2 - mean^2 (var of S)
                a2 = coefp.tile([CH, 1], F32, tag=f"a2{conv}{oc}", name=f"a2{conv}{oc}")
                nc.vector.tensor_mul(a2[:], al, al)
                nc.vector.tensor_mul(var[:], var[:], a2[:])       # var of alpha*S
                sd = coefp.tile([CH, 1], F32, tag=f"sd{conv}{oc}", name=f"sd{conv}{oc}")
                nc.scalar.activation(
                    out=sd[:], in_=var[:],
                    func=mybir.ActivationFunctionType.Sqrt,
                    bias=epst[:], scale=1.0,
                )
                r = coefp.tile([CH, 1], F32, tag=f"r{conv}{oc}", name=f"r{conv}{oc}")
                nc.vector.reciprocal(r[:], sd[:])                 # rstd
                a_t = coefp.tile([CH, 1], F32, tag=f"A{conv}{oc}", name=f"A{conv}{oc}")
                nc.vector.tensor_mul(a_t[:], al, ga)
                nc.vector.tensor_mul(a_t[:], a_t[:], r[:])        # A = alpha*g*rstd
                b_t = coefp.tile([CH, 1], F32, tag=f"B{conv}{oc}", name=f"B{conv}{oc}")
                nc.vector.tensor_mul(b_t[:], m[:], a_t[:])        # mean_S * A
                nc.vector.tensor_sub(b_t[:], be, b_t[:])          # B = beta - mean_S*A
                A.append(a_t)
                B.append(b_t)
            return A, B

        def stats_collective(stats, conv):
            """Per-tile (sum, sumsq) columns -> totals -> AllReduce -> coef tiles."""
            ccs = coefp.tile([CH, 4], F32, tag=f"ccs{conv}", name=f"ccs{conv}")
            n_full = float(NT * n_img * NYC)
            for oc in range(2):
                bnb, gs, gq = stats[oc]
                mv = coefp.tile([CH, 2], F32, tag=f"mv{conv}{oc}", name=f"mv{conv}{oc}")
                nc.vector.bn_aggr(
                    out=mv[:], in_=bnb[:].rearrange("p a s -> p (a s)")
                )
                tots = coefp.tile([CH, 4], F32, tag=f"tots{conv}{oc}", name=f"tots{conv}{oc}")
                # full sum = mean * n_full ; full sumsq = (var + mean^2) * n_full
                nc.vector.tensor_scalar_mul(tots[:, 0:1], mv[:, 0:1], n_full)
                nc.vector.tensor_mul(tots[:, 1:2], mv[:, 0:1], mv[:, 0:1])
                nc.vector.tensor_add(tots[:, 1:2], tots[:, 1:2], mv[:, 1:2])
                nc.vector.tensor_scalar_mul(tots[:, 1:2], tots[:, 1:2], n_full)
                # garbage totals
                nc.vector.tensor_reduce(
                    out=tots[:, 2:3], in_=gs[:],
                    axis=mybir.AxisListType.X, op=mybir.AluOpType.add,
                )
                nc.vector.tensor_reduce(
                    out=tots[:, 3:4], in_=gq[:],
                    axis=mybir.AxisListType.X, op=mybir.AluOpType.add,
                )
                nc.vector.tensor_sub(
                    ccs[:, 2 * oc : 2 * oc + 1], tots[:, 0:1], tots[:, 2:3]
                )
                nc.vector.tensor_sub(
                    ccs[:, 2 * oc + 1 : 2 * oc + 2], tots[:, 1:2], tots[:, 3:4]
                )
            cci = dramp.tile([CH, 4], F32, tag=f"cci{conv}", name=f"cci{conv}")
            cco = dramp.tile([CH, 4], F32, tag=f"cco{conv}", name=f"cco{conv}")
            nc.sync.dma_start(out=cci[:], in_=ccs[:])
            if USE_COLLECTIVE:
                nc.gpsimd.collective_compute(
                    "AllReduce",
                    mybir.AluOpType.add,
                    replica_groups=[list(range(n_cores))],
                    ins=[cci[:].opt()],
                    outs=[cco[:].opt()],
                )
            else:
                nc.sync.dma_start(out=cco[:], in_=cci[:])
            ccg = coefp.tile([CH, 4], F32, tag=f"ccg{conv}", name=f"ccg{conv}")
            nc.sync.dma_start(out=ccg[:], in_=cco[:])
            return make_coefs(ccg, conv)

        # ---------------- phase 1: binarize x into padded planes -------------
        planes1 = []
        for n in range(n_img):
            pl = planep.tile([CH, 2, PLANE_F], pdt, tag="plane", name="plane")
            nc.gpsimd.memset(pl[:], 0)
            for j in range(2):
                for yc in range(NYC):
                    xs = xpool.tile([CH, RPT * W], F32, tag="xs", name="xs")
                    nc.sync.dma_start(
                        out=xs[:],
                        in_=x_in[
                            n, j * CH : (j + 1) * CH, yc * RPT : (yc + 1) * RPT, :
                        ].rearrange("c h w -> c (h w)"),
                    )
                    dst = (
                        pl[:, j, 59 + yc * RPT * PW : 59 + (yc + 1) * RPT * PW]
                        .rearrange("p (y x) -> p y x", x=PW)[:, :, 0:W]
                    )
                    nc.scalar.activation(
                        out=dst,
                        in_=xs[:].rearrange("p (y x) -> p y x", x=W),
                        func=mybir.ActivationFunctionType.Sign,
                    )
            planes1.append(pl)

        # ---------------- phase 2: conv1 + local stats -----------------------
        stats1 = [
            (
                statsp.tile([CH, n_img * NYC, 6], F32, tag=f"st1b_{oc}", name=f"st1b_{oc}"),
                statsp.tile([CH, n_img], F32, tag=f"st1gs_{oc}", name=f"st1gs_{oc}"),
                statsp.tile([CH, n_img], F32, tag=f"st1gq_{oc}", name=f"st1gq_{oc}"),
            )
            for oc in range(2)
        ] if stage >= 2 else None
        s1 = {}
        for n in range(n_img if stage >= 2 else 0):
            for oc in range(2):
                s = spool.tile([CH, NYC, NT], F16, tag="simg", name="simg")
                s1[(n, oc)] = s
                for yc in range(NYC):
                    ps = psum.tile([CH, NT], F32, tag="ps", name="ps")
                    conv_tile(w1t, planes1[n], oc, yc, ps)
                    nc.scalar.copy(out=s[:, yc, :], in_=ps[:])
                    t_idx = n * NYC + yc
                    bnb, gs, gq = stats1[oc]
                    nc.vector.bn_stats(out=bnb[:, t_idx, :], in_=s[:, yc, :])
                # per-image garbage-column correction (cols 56,57 of each row)
                gv = s[:].rearrange("p y (r x) -> p y r x", x=PW)[:, :, :, W:PW]
                nc.vector.tensor_reduce(
                    out=stats1[oc][1][:, n : n + 1], in_=gv,
                    axis=mybir.AxisListType.XYZ, op=mybir.AluOpType.add,
                )
                gsq = scrp.tile([CH, NYC, RPT, 2], F32, tag="gsq", name="gsq")
                nc.vector.tensor_mul(gsq[:], gv, gv)
                nc.vector.tensor_reduce(
                    out=stats1[oc][2][:, n : n + 1], in_=gsq[:],
                    axis=mybir.AxisListType.XYZ, op=mybir.AluOpType.add,
                )

        A1, B1 = stats_collective(stats1, 0) if stage >= 3 else (None, None)

        # ---------------- phase 3: binarize BN1 output into planes -----------
        planes2 = []
        for n in range(n_img if stage >= 4 else 0):
            pl = planep.tile([CH, 2, PLANE_F], pdt, tag="plane", name="plane")
            nc.gpsimd.memset(pl[:], 0)
            for j in range(2):
                dst = pl[:, j, 59 : 59 + NYC * NT]
                nc.scalar.activation(
                    out=dst,
                    in_=s1[(n, j)][:].rearrange("p y t -> p (y t)"),
                    func=mybir.ActivationFunctionType.Sign,
                    scale=A1[j][:],
                    bias=B1[j][:],
                )
                # zero the per-row garbage pairs (cols 56,57 of each output row,
                # which the shifted copy landed on pad positions)
                gv = (
                    pl[:, j, 115 : 115 + H * PW]
                    .rearrange("p (y x) -> p y x", x=PW)[:, :, 0:2]
                )
                nc.vector.memset(gv, 0)
            planes2.append(pl)

        # ---------------- phase 4: conv2 + local stats -----------------------
        stats2 = [
            (
                statsp.tile([CH, n_img * NYC, 6], F32, tag=f"st2b_{oc}", name=f"st2b_{oc}"),
                statsp.tile([CH, n_img], F32, tag=f"st2gs_{oc}", name=f"st2gs_{oc}"),
                statsp.tile([CH, n_img], F32, tag=f"st2gq_{oc}", name=f"st2gq_{oc}"),
            )
            for oc in range(2)
        ] if stage >= 5 else None
        s2 = {}
        for n in range(n_img if stage >= 5 else 0):
            for oc in range(2):
                s = spool.tile([CH, NYC, NT], F16, tag="simg", name="simg")
                s2[(n, oc)] = s
                for yc in range(NYC):
                    ps = psum.tile([CH, NT], F32, tag="ps", name="ps")
                    conv_tile(w2t, planes2[n], oc, yc, ps)
                    nc.scalar.copy(out=s[:, yc, :], in_=ps[:])
                    t_idx = n * NYC + yc
                    bnb, gs, gq = stats2[oc]
                    nc.vector.bn_stats(out=bnb[:, t_idx, :], in_=s[:, yc, :])
                # per-image garbage-column correction (cols 56,57 of each row)
                gv = s[:].rearrange("p y (r x) -> p y r x", x=PW)[:, :, :, W:PW]
                nc.vector.tensor_reduce(
                    out=stats2[oc][1][:, n : n + 1], in_=gv,
                    axis=mybir.AxisListType.XYZ, op=mybir.AluOpType.add,
                )
                gsq = scrp.tile([CH, NYC, RPT, 2], F32, tag="gsq", name="gsq")
                nc.vector.tensor_mul(gsq[:], gv, gv)
                nc.vector.tensor_reduce(
                    out=stats2[oc][2][:, n : n + 1], in_=gsq[:],
                    axis=mybir.AxisListType.XYZ, op=mybir.AluOpType.add,
                )

        A2, B2 = stats_collective(stats2, 1) if stage >= 5 else (None, None)

        # ---------------- phase 5: affine + residual + store -----------------
        YGROUPS = [(0, 2), (2, 2), (4, 2), (6, 1)]
        for n in range(n_img if stage >= 6 else 0):
            for oc in range(2):
                for y0, k in YGROUPS:
                    rows = k * RPT
                    xid = xidp.tile([CH, 2 * RPT, W], F32, tag="xid", name="xid")
                    nc.sync.dma_start(
                        out=xid[:, 0:rows, :],
                        in_=x_in[
                            n, oc * CH : (oc + 1) * CH,
                            y0 * RPT : y0 * RPT + rows, :,
                        ],
                    )
                    fin = fpool.tile([CH, 2 * RPT, W], F32, tag="fin", name="fin")
                    nc.scalar.activation(
                        out=fin[:, 0:rows, :],
                        in_=s2[(n, oc)][:, y0 : y0 + k]
                        .rearrange("p y (r x) -> p (y r) x", x=PW)[:, :, 0:W],
                        func=mybir.ActivationFunctionType.Identity,
                        scale=A2[oc][:],
                        bias=B2[oc][:],
                    )
                    nc.vector.tensor_add(
                        fin[:, 0:rows, :], fin[:, 0:rows, :], xid[:, 0:rows, :]
                    )
                    nc.sync.dma_start(
                        out=out_t[
                            n, oc * CH : (oc + 1) * CH,
                            y0 * RPT : y0 * RPT + rows, :,
                        ],
                        in_=fin[:, 0:rows, :],
                    )

    if not nc.is_finalized():
        nc.finalize()
    return nc


def pack_weights(w):
    """w [256,256,3,3] f32 -> [128(c), 9(off), 2(oc), 2(j), 128(o)] sign in fp8/bf16."""
    s = np.sign(w).astype(np.float32)          # [O, I, 3, 3]
    s = s.reshape(2, CH, 2, CH, 3, 3)          # [oc, o, j, c, dy, dx]
    s = s.transpose(3, 4, 5, 0, 2, 1)          # [c, dy, dx, oc, j, o]
    s = np.ascontiguousarray(s.reshape(CH, 9, 2, 2, CH))
    return s.astype(F8NP if USE_FP8 else BF16NP)


def pack_aux(w1, g1, b1, w2, g2, b2):
    aux = np.zeros((CH, 12), np.float32)
    for conv, (w, g, b) in enumerate(((w1, g1, b1), (w2, g2, b2))):
        alpha = np.abs(w).mean(axis=(1, 2, 3), dtype=np.float32)  # [256]
        for oc in range(2):
            aux[:, conv * 6 + 0 + oc] = alpha[oc * CH : (oc + 1) * CH]
            aux[:, conv * 6 + 2 + oc] = g[oc * CH : (oc + 1) * CH]
            aux[:, conv * 6 + 4 + oc] = b[oc * CH : (oc + 1) * CH]
    return aux


_NC_CACHE = {}


def _ensure_ntff_hook():
    """Register the axon NTFF profiling hook if the image's antenv lacks it."""
    import types

    try:
        from antenv.axon_hooks import get_axon_ntff_profile_hook  # noqa: F401
        return
    except ImportError:
        pass
    try:
        import antenv
        from trn_agent_boot.trn_boot import _ntff_profile_via_ctypes

        hook = _ntff_profile_via_ctypes("/opt/axon/libaxon_pjrt.so")
        mod = types.ModuleType("antenv.axon_hooks")
        mod._hook = hook

        def set_axon_ntff_profile_hook(h):
            mod._hook = h

        def get_axon_ntff_profile_hook():
            return mod._hook

        mod.set_axon_ntff_profile_hook = set_axon_ntff_profile_hook
        mod.get_axon_ntff_profile_hook = get_axon_ntff_profile_hook
        sys.modules["antenv.axon_hooks"] = mod
        antenv.axon_hooks = mod
    except Exception:
        pass


def kernel(x, w1, g1, b1, w2, g2, b2, _trace=False):
    x = np.asarray(x, np.float32)
    n_total = x.shape[0]
    assert n_total == N_CORES * N_IMG, x.shape
    key = (N_IMG, N_CORES)
    if key not in _NC_CACHE:
        _NC_CACHE[key] = build_nc(N_IMG, N_CORES)
    nc = _NC_CACHE[key]

    w1p = pack_weights(np.asarray(w1, np.float32))
    w2p = pack_weights(np.asarray(w2, np.float32))
    aux = pack_aux(
        np.asarray(w1, np.float32), np.asarray(g1, np.float32), np.asarray(b1, np.float32),
        np.asarray(w2, np.float32), np.asarray(g2, np.float32), np.asarray(b2, np.float32),
    )

    if _trace:
        _ensure_ntff_hook()
    in_maps = [
        {
            "x": np.ascontiguousarray(x[c * N_IMG : (c + 1) * N_IMG]),
            "w1p": w1p,
            "w2p": w2p,
            "aux": aux,
        }
        for c in range(N_CORES)
    ]
    res = run_bass_kernel_spmd(
        nc, in_maps, core_ids=list(range(N_CORES)), trace=_trace
    )
    out = np.concatenate([r["out"] for r in res.results], axis=0).astype(np.float32)
    if _trace:
        return out, res
    return out



# revision 11
# speedup vs baseline: 1.2854x; 1.2854x over previous
"""Bass/Trainium2 kernel for nn_BitwiseBasicBlock.

Computes (reference semantics, NCHW):
    out1 = BN(conv3x3(sign(x), sign(w1)*alpha1), g1, b1)     # training-mode BN
    out2 = BN(conv3x3(sign(out1), sign(w2)*alpha2), g2, b2)
    out  = out2 + x

Strategy (v2):
  - Data-parallel over batch: 32 images -> 8 cores x 4 images. Weights replicated.
  - Sync-BN via AllReduce of per-core (sum, sumsq); split per 128-channel half
    (oc) and ordered oc-outer so each half's AllReduce overlaps the other
    half's convolutions.
  - x kept SBUF-resident in bf16 (loaded once, reused for sign and residual);
    output stored bf16 and widened on host. Halves all HBM traffic.
  - Conv operands are +-1: 3x3 conv = 9 accumulated fp8 DoubleRow matmuls over
    a zero-padded 58-wide plane, k-outer over 7 PSUM banks so weight loads
    amortize across row-chunks.
  - Plane borders zeroed with 3 small strided memsets instead of full-plane
    memset; BN stats read strided 448-elem views (no garbage correction).
  - Tail fuses BN affine + residual into one DVE affine_then_add per tile.
"""

import os
import sys

import numpy as np

for _p in ("/opt/trn_rl_repo",):
    if _p not in sys.path and os.path.isdir(_p):
        sys.path.insert(0, _p)

import ml_dtypes
from contextlib import ExitStack

import concourse.bass as bass
import concourse.tile as tile
from concourse import bacc
from concourse import mybir
from concourse.bass_utils import run_bass_kernel_spmd

F32 = mybir.dt.float32
F16 = mybir.dt.float16
BF16 = mybir.dt.bfloat16
F8 = mybir.dt.float8e4
F8NP = ml_dtypes.float8_e4m3
BF16NP = ml_dtypes.bfloat16

EPS = 1e-5
H = W = 56
HWP = H * W           # 3136
PW = H + 2            # padded width
RPT = 8               # output rows per psum tile
NYC = H // RPT        # 7 row-chunks
NT = RPT * PW         # 464 <= 512 (one PSUM bank)
CH = 128              # channel chunk (partition dim)
PLANE_F = 3488        # per-chunk padded plane size; >= 59*58
OFFS = [(dy, dx) for dy in range(3) for dx in range(3)]

USE_COLLECTIVE = True # AllReduce for BN stats (False: local copy, 1-core only)

N_CORES = 8
N_IMG = 4             # images per core on HW


def build_nc(n_img, n_cores):
    nc = bacc.Bacc("TRN2", target_bir_lowering=False)
    xb = nc.dram_tensor("xb", [n_img, 2, CH, HWP], BF16, kind="ExternalInput")
    w1p = nc.dram_tensor("w1p", [CH, 9, 2, 2, CH], F8, kind="ExternalInput")
    w2p = nc.dram_tensor("w2p", [CH, 9, 2, 2, CH], F8, kind="ExternalInput")
    # aux cols per conv: (alpha, gamma, beta) x oc
    aux = nc.dram_tensor("aux", [CH, 12], F32, kind="ExternalInput")
    out_t = nc.dram_tensor("out", [n_img, 2, CH, HWP], BF16, kind="ExternalOutput")

    count_g = float(n_img * n_cores * HWP)  # global per-channel element count

    with ExitStack() as ctx:
        tc = ctx.enter_context(tile.TileContext(nc))
        singles = ctx.enter_context(tc.tile_pool(name="singles", bufs=1))
        planep = ctx.enter_context(tc.tile_pool(name="planep", bufs=5))
        spool = ctx.enter_context(tc.tile_pool(name="spool", bufs=10))
        statsp = ctx.enter_context(tc.tile_pool(name="statsp", bufs=1))
        coefp = ctx.enter_context(tc.tile_pool(name="coefp", bufs=1))
        psum = ctx.enter_context(tc.tile_pool(name="psum", bufs=8, space="PSUM"))
        finp = ctx.enter_context(tc.tile_pool(name="finp", bufs=4))
        dramp = ctx.enter_context(tc.tile_pool(name="dramp", bufs=8, space="DRAM"))

        w1t = singles.tile([CH, 9, 2, 2, CH], F8)
        nc.sync.dma_start(out=w1t[:], in_=w1p[:])
        w2t = singles.tile([CH, 9, 2, 2, CH], F8)
        nc.sync.dma_start(out=w2t[:], in_=w2p[:])
        auxt = singles.tile([CH, 12], F32)
        nc.sync.dma_start(out=auxt[:], in_=aux[:])
        epst = singles.tile([CH, 1], F32)
        nc.vector.memset(epst[:], EPS)

        # resident bf16 input (sign source + residual)
        xbt = singles.tile([CH, n_img, 2, HWP], BF16)
        for n in range(n_img):
            for j in range(2):
                nc.sync.dma_start(out=xbt[:, n, j, :], in_=xb[n, j])

        # ---------------- helpers ------------------------------------------
        def plane_border_pre(pl, j):
            """Top pad row + row-1 col 0 (offsets [0,59))."""
            nc.gpsimd.memset(pl[:, j, 0:59], 0)

        def plane_border_post(pl, j):
            """(col57,row r)+(col0,row r+1) pairs rows 1..55, then bottom
            pad rows (r56c57 onward). Emitted after any overlapping write."""
            bv = pl[:, j, 115 : 115 + 55 * PW].rearrange(
                "p (r x) -> p r x", x=PW
            )[:, :, 0:2]
            nc.gpsimd.memset(bv, 0)
            nc.gpsimd.memset(pl[:, j, 3305:PLANE_F], 0)

        def make_plane1(n):
            pl = planep.tile([CH, 2, PLANE_F], F8, tag="plane", name="plane")
            for j in range(2):
                plane_border_pre(pl, j)
                plane_border_post(pl, j)
                for half in range(2):
                    r0 = half * 28
                    dst = pl[:, j, 59 + r0 * PW : 59 + (r0 + 28) * PW].rearrange(
                        "p (y x) -> p y x", x=PW
                    )[:, :, 0:W]
                    src = xbt[:, n, j, r0 * W : (r0 + 28) * W].rearrange(
                        "p (y x) -> p y x", x=W
                    )
                    nc.scalar.activation(
                        out=dst, in_=src,
                        func=mybir.ActivationFunctionType.Sign,
                    )
            return pl

        def conv_group(wt, plane, oc, s):
            """k-outer 3x3 conv: 9 weight loads, 63 matmuls into 7 banks."""
            pss = [
                psum.tile([CH, NT], F32, tag="ps", name="ps")
                for _ in range(NYC)
            ]
            for k in range(9):
                dy, dx = OFFS[k]
                for yc in range(NYC):
                    off = yc * RPT * PW + dy * PW + dx
                    nc.tensor.matmul(
                        out=pss[yc][:],
                        lhsT=wt[:, k, oc],
                        rhs=plane[:, :, off : off + NT],
                        start=(k == 0),
                        stop=(k == 8),
                        perf_mode=mybir.MatmulPerfMode.DoubleRow,
                    )
            for yc in range(NYC):
                nc.scalar.copy(out=s[:, yc, :], in_=pss[yc][:])
            # zero the 2 garbage columns per row so they drop out of the
            # BN sums (and sign2 writes sign(B) there, cleaned by the
            # plane border memsets)
            gv = s[:].rearrange("p y (r x) -> p y r x", x=PW)[:, :, :, W:PW]
            nc.vector.memset(gv, 0)

        def stats_for(s, bnb, n):
            for yc in range(NYC):
                nc.vector.bn_stats(out=bnb[:, n * NYC + yc, :], in_=s[:, yc, :])

        def stats_trigger(bnb, conv, oc):
            """bnb [CH, n_img*NYC, 6] -> (sum, sumsq) -> AllReduce; returns
            the post-reduce SBUF tile (filled once the collective lands)."""
            tag = f"c{conv}{oc}"
            n_loc = float(n_img * NYC * NT)  # garbage cols are zeroed
            mv = coefp.tile([CH, 2], F32, tag=f"mv{tag}", name=f"mv{tag}")
            nc.vector.bn_aggr(out=mv[:], in_=bnb[:].rearrange("p a s -> p (a s)"))
            ccs = coefp.tile([CH, 2], F32, tag=f"ccs{tag}", name=f"ccs{tag}")
            # sum = mean * n_loc ; sumsq = (var + mean^2) * n_loc
            nc.vector.tensor_scalar_mul(ccs[:, 0:1], mv[:, 0:1], n_loc)
            nc.vector.tensor_mul(ccs[:, 1:2], mv[:, 0:1], mv[:, 0:1])
            nc.vector.tensor_add(ccs[:, 1:2], ccs[:, 1:2], mv[:, 1:2])
            nc.vector.tensor_scalar_mul(ccs[:, 1:2], ccs[:, 1:2], n_loc)
            cci = dramp.tile([CH, 2], F32, tag=f"cci{tag}", name=f"cci{tag}")
            cco = dramp.tile([CH, 2], F32, tag=f"cco{tag}", name=f"cco{tag}")
            nc.sync.dma_start(out=cci[:], in_=ccs[:])
            if USE_COLLECTIVE:
                nc.gpsimd.collective_compute(
                    "AllReduce",
                    mybir.AluOpType.add,
                    replica_groups=[list(range(n_cores))],
                    ins=[cci[:].opt()],
                    outs=[cco[:].opt()],
                )
            else:
                nc.sync.dma_start(out=cco[:], in_=cci[:])
            ccg = coefp.tile([CH, 2], F32, tag=f"ccg{tag}", name=f"ccg{tag}")
            nc.sync.dma_start(out=ccg[:], in_=cco[:])
            return ccg

        def coef_finish(ccg, conv, oc):
            """Global (sum, sumsq) -> per-channel A, B with
            BN(alpha*S)*g + b == S*A + B. Emit well after stats_trigger so
            the scalar Sqrt does not head-of-line block PSUM drains."""
            tag = f"c{conv}{oc}"
            al = auxt[:, conv * 6 + 0 + oc : conv * 6 + 0 + oc + 1]
            ga = auxt[:, conv * 6 + 2 + oc : conv * 6 + 2 + oc + 1]
            be = auxt[:, conv * 6 + 4 + oc : conv * 6 + 4 + oc + 1]
            m = coefp.tile([CH, 1], F32, tag=f"m{tag}", name=f"m{tag}")
            nc.vector.tensor_scalar_mul(m[:], ccg[:, 0:1], 1.0 / count_g)
            e2 = coefp.tile([CH, 1], F32, tag=f"e2{tag}", name=f"e2{tag}")
            nc.vector.tensor_scalar_mul(e2[:], ccg[:, 1:2], 1.0 / count_g)
            var = coefp.tile([CH, 1], F32, tag=f"var{tag}", name=f"var{tag}")
            nc.vector.tensor_mul(var[:], m[:], m[:])
            nc.vector.tensor_sub(var[:], e2[:], var[:])      # var of S
            a2t = coefp.tile([CH, 1], F32, tag=f"a2{tag}", name=f"a2{tag}")
            nc.vector.tensor_mul(a2t[:], al, al)
            nc.vector.tensor_mul(var[:], var[:], a2t[:])     # var of alpha*S
            sd = coefp.tile([CH, 1], F32, tag=f"sd{tag}", name=f"sd{tag}")
            nc.scalar.activation(
                out=sd[:], in_=var[:],
                func=mybir.ActivationFunctionType.Sqrt,
                bias=epst[:], scale=1.0,
            )
            r = coefp.tile([CH, 1], F32, tag=f"r{tag}", name=f"r{tag}")
            nc.vector.reciprocal(r[:], sd[:])
            a_t = coefp.tile([CH, 1], F32, tag=f"A{tag}", name=f"A{tag}")
            nc.vector.tensor_mul(a_t[:], al, ga)
            nc.vector.tensor_mul(a_t[:], a_t[:], r[:])       # A = alpha*g*rstd
            b_t = coefp.tile([CH, 1], F32, tag=f"B{tag}", name=f"B{tag}")
            nc.vector.tensor_mul(b_t[:], m[:], a_t[:])
            nc.vector.tensor_sub(b_t[:], be, b_t[:])         # B = beta - mean*A
            return a_t, b_t

        # ---------------- phase 1 + conv1 (oc-outer) -----------------------
        planes1 = {0: make_plane1(0)}
        # warm the Sqrt activation table off the critical path
        warm = coefp.tile([CH, 1], F32, tag="warm", name="warm")
        nc.scalar.activation(
            out=warm[:], in_=epst[:],
            func=mybir.ActivationFunctionType.Sqrt,
        )

        stats1 = [
            statsp.tile([CH, n_img * NYC, 6], F32, tag=f"st1_{oc}", name=f"st1_{oc}")
            for oc in range(2)
        ]
        s1 = {}
        A1 = [None, None]
        B1 = [None, None]
        planes2 = {}

        def sign2_scalar(n, j):
            """binarize BN1 output half j of image n into planes2[n] (no
            border cleanup — plane_border_post(planes2[n], j) must be
            emitted separately, after any AllReduce trigger it could block)."""
            pl = planes2[n]
            for half in range(2):
                dst = pl[:, j, 59 + half * 1624 : 59 + (half + 1) * 1624]
                src = s1[(n, j)][:].rearrange("p y t -> p (y t)")[
                    :, half * 1624 : (half + 1) * 1624
                ]
                nc.scalar.activation(
                    out=dst, in_=src,
                    func=mybir.ActivationFunctionType.Sign,
                    scale=A1[j][:], bias=B1[j][:],
                )

        ccg1 = [None, None]
        late_n = max(0, n_img - 2)
        for oc in range(2):
            for n in range(n_img):
                if oc == 0 and n + 1 < n_img:
                    planes1[n + 1] = make_plane1(n + 1)
                if oc == 1 and n == late_n:
                    # coefficients for oc=0: its AllReduce overlapped the
                    # first two oc=1 conv groups
                    A1[0], B1[0] = coef_finish(ccg1[0], 0, 0)
                s = spool.tile([CH, NYC, NT], F16, tag="simg", name="simg")
                s1[(n, oc)] = s
                conv_group(w1t, planes1[n], oc, s)
                stats_for(s, stats1[oc], n)
                if oc == 1 and n == late_n:
                    # sign2 for j=0 of all images runs during the last group
                    for nn in range(n_img):
                        pl = planep.tile(
                            [CH, 2, PLANE_F], F8, tag="plane", name="plane"
                        )
                        planes2[nn] = pl
                        plane_border_pre(pl, 0)
                        plane_border_pre(pl, 1)
                        sign2_scalar(nn, 0)
            ccg1[oc] = stats_trigger(stats1[oc], 0, oc)

        # j=0 border cleanup, queued on gpsimd after the oc=1 AllReduce trigger
        for n in range(n_img):
            plane_border_post(planes2[n], 0)
        A1[1], B1[1] = coef_finish(ccg1[1], 0, 1)

        # ---------------- sign2 j=1 + conv2 (oc-outer) ---------------------
        stats2 = [
            statsp.tile([CH, n_img * NYC, 6], F32, tag=f"st2_{oc}", name=f"st2_{oc}")
            for oc in range(2)
        ]
        s2 = {}
        A2 = [None, None]
        B2 = [None, None]

        sign2_scalar(0, 1)
        plane_border_post(planes2[0], 1)

        def tail(n, oc):
            fin = finp.tile([CH, H, W], BF16, tag="fin", name="fin")
            in0 = s2[(n, oc)][:].rearrange("p y (r x) -> p (y r) x", x=PW)[
                :, :, 0:W
            ]
            in1 = xbt[:, n, oc, :].rearrange("p (y x) -> p y x", x=W)
            nc.vector.tensor_scalar(
                out=fin[:], in0=in0,
                scalar1=A2[oc][:], scalar2=B2[oc][:],
                op0=mybir.AluOpType.mult, op1=mybir.AluOpType.add,
            )
            nc.vector.tensor_add(fin[:], fin[:], in1)
            nc.sync.dma_start(
                out=out_t[n, oc].rearrange("p (y x) -> p y x", x=W),
                in_=fin[:],
            )

        ccg2 = [None, None]
        for oc in range(2):
            for n in range(n_img):
                if oc == 0 and n + 1 < n_img:
                    sign2_scalar(n + 1, 1)
                    plane_border_post(planes2[n + 1], 1)
                if oc == 1 and n == late_n:
                    A2[0], B2[0] = coef_finish(ccg2[0], 1, 0)
                s = spool.tile([CH, NYC, NT], F16, tag="simg", name="simg")
                s2[(n, oc)] = s
                conv_group(w2t, planes2[n], oc, s)
                stats_for(s, stats2[oc], n)
                if oc == 1 and n == late_n:
                    # oc=0 tail (affine+residual+store) overlaps both the
                    # last oc=1 conv group and the oc=1 AllReduce
                    for nn in range(n_img):
                        tail(nn, 0)
            ccg2[oc] = stats_trigger(stats2[oc], 1, oc)
        A2[1], B2[1] = coef_finish(ccg2[1], 1, 1)
        for n in range(n_img):
            tail(n, 1)

    if not nc.is_finalized():
        nc.finalize()
    return nc


def pack_weights(w):
    """w [256,256,3,3] f32 -> [128(c), 9(off), 2(oc), 2(j), 128(o)] sign fp8."""
    s = np.sign(w).astype(np.float32)          # [O, I, 3, 3]
    s = s.reshape(2, CH, 2, CH, 3, 3)          # [oc, o, j, c, dy, dx]
    s = s.transpose(3, 4, 5, 0, 2, 1)          # [c, dy, dx, oc, j, o]
    s = np.ascontiguousarray(s.reshape(CH, 9, 2, 2, CH))
    return s.astype(F8NP)


def pack_aux(w1, g1, b1, w2, g2, b2):
    aux = np.zeros((CH, 12), np.float32)
    for conv, (w, g, b) in enumerate(((w1, g1, b1), (w2, g2, b2))):
        alpha = np.abs(w).mean(axis=(1, 2, 3), dtype=np.float32)  # [256]
        for oc in range(2):
            aux[:, conv * 6 + 0 + oc] = alpha[oc * CH : (oc + 1) * CH]
            aux[:, conv * 6 + 2 + oc] = g[oc * CH : (oc + 1) * CH]
            aux[:, conv * 6 + 4 + oc] = b[oc * CH : (oc + 1) * CH]
    return aux


_NC_CACHE = {}


def _ensure_ntff_hook():
    """Register the axon NTFF profiling hook if the image's antenv lacks it."""
    import types

    try:
        from antenv.axon_hooks import get_axon_ntff_profile_hook  # noqa: F401
        return
    except ImportError:
        pass
    try:
        import antenv
        from trn_agent_boot.trn_boot import _ntff_profile_via_ctypes

        hook = _ntff_profile_via_ctypes("/opt/axon/libaxon_pjrt.so")
        mod = types.ModuleType("antenv.axon_hooks")
        mod._hook = hook

        def set_axon_ntff_profile_hook(h):
            mod._hook = h

        def get_axon_ntff_profile_hook():
            return mod._hook

        mod.set_axon_ntff_profile_hook = set_axon_ntff_profile_hook
        mod.get_axon_ntff_profile_hook = get_axon_ntff_profile_hook
        sys.modules["antenv.axon_hooks"] = mod
        antenv.axon_hooks = mod
    except Exception:
        pass


def kernel(x, w1, g1, b1, w2, g2, b2, _trace=False):
    x = np.asarray(x, np.float32)
    n_total = x.shape[0]
    assert n_total == N_CORES * N_IMG, x.shape
    key = (N_IMG, N_CORES)
    if key not in _NC_CACHE:
        _NC_CACHE[key] = build_nc(N_IMG, N_CORES)
    nc = _NC_CACHE[key]

    w1p = pack_weights(np.asarray(w1, np.float32))
    w2p = pack_weights(np.asarray(w2, np.float32))
    aux = pack_aux(
        np.asarray(w1, np.float32), np.asarray(g1, np.float32), np.asarray(b1, np.float32),
        np.asarray(w2, np.float32), np.asarray(g2, np.float32), np.asarray(b2, np.float32),
    )
    xbf = x.reshape(N_CORES, N_IMG, 2, CH, HWP).astype(BF16NP)

    if _trace:
        _ensure_ntff_hook()
    in_maps = [
        {
            "xb": np.ascontiguousarray(xbf[c]),
            "w1p": w1p,
            "w2p": w2p,
            "aux": aux,
        }
        for c in range(N_CORES)
    ]
    res = run_bass_kernel_spmd(
        nc, in_maps, core_ids=list(range(N_CORES)), trace=_trace
    )
    out = np.concatenate(
        [
            r["out"].astype(np.float32).reshape(N_IMG, 2 * CH, H, W)
            for r in res.results
        ],
        axis=0,
    )
    if _trace:
        return out, res
    return out
